# revision 1
# baseline (speedup 1.0000x reference)
"""TRN2 Bass kernel for nn_AttentionModel_46823733461774.

Gemma3n-style attention block: qkv projection, q/k/v RMS-norm, RoPE on q/k,
GQA causal attention (no scaling; q_norm replaces 1/sqrt(d)), output proj.

Shapes (hardcoded): B=2, S=2048, D=2048, H=8, KV=2, DH=256.

Sharding over 8 cores: core c -> batch b=c//4, q-heads {2j, 2j+1} (j=c%4),
kv-head j//2.  Each core computes the projections for its batch/heads
(token-major), norms+RoPE, causal attention for its 2 heads, and a partial
output projection attn_heads @ wo_slice^T.  Host sums the 4 partials per
batch.  cos/sin replicated.

All matmuls in fp16 (same PE throughput as bf16, 8x the mantissa accuracy);
softmax statistics and accumulations in fp32.
"""

import os
import numpy as np
import ml_dtypes

import concourse.bass as bass
import concourse.mybir as mybir
import concourse.tile as tile
from concourse import bacc
from concourse import bass_utils

B, S, D = 2, 2048, 2048
H, KV, DH = 8, 2, 256
EPS = 1e-6
NEG = -1e30
P = 128
TT = S // P      # 16 token tiles
DT = D // P      # 16 contraction tiles
NH = 2           # heads per core
KC = 512         # key chunk (scores free dim)

# matmul dtype mode: "f16" | "bf16" | "f32"
MODE = os.environ.get("KERNEL_MODE", "f16")
# repeat the body N times inside the NEFF (for wall-clock HW timing)
ITERS = int(os.environ.get("KERNEL_ITERS", "1"))

_cache = {}


def _np_md():
    if MODE == "bf16":
        return ml_dtypes.bfloat16
    if MODE == "f16":
        return np.float16
    return np.float32


def _bir_md():
    if MODE == "bf16":
        return mybir.dt.bfloat16
    if MODE == "f16":
        return mybir.dt.float16
    return mybir.dt.float32


def _build_program():
    f32 = mybir.dt.float32
    md = _bir_md()
    Alu = mybir.AluOpType
    Act = mybir.ActivationFunctionType
    X = mybir.AxisListType.X
    XY = mybir.AxisListType.XY

    nc = bacc.Bacc("TRN2", target_bir_lowering=False, debug=False, num_devices=8)

    # fp16 buffers hang at the PJRT/axon boundary -> declare 2-byte inputs
    # as uint16 and bitcast to the matmul dtype on the DRAM APs.
    io2 = mybir.dt.uint16 if mybir.dt.size(md) == 2 else md
    def _in2(name, shape):
        ap = nc.dram_tensor(name, shape, io2, kind="ExternalInput").ap()
        return ap.bitcast(md) if io2 != md else ap
    xT_d = _in2("xT", [D, S])
    wqT_d = _in2("wqT", [D, NH * DH])
    wkvT_d = _in2("wkvT", [D, 2 * DH])
    woT2_d = _in2("woT2", [NH * DH, D])
    cos_d = _in2("cosb", [S, DH])
    sin_d = _in2("sinb", [S, DH])
    qw_d = nc.dram_tensor("qw", [P, DH], f32, kind="ExternalInput").ap()
    kw_d = nc.dram_tensor("kw", [P, DH], f32, kind="ExternalInput").ap()
    trimask_d = nc.dram_tensor("trimask", [P, P], f32, kind="ExternalInput").ap()
    ident_d = _in2("ident", [P, P])
    out_d = nc.dram_tensor("out", [S, D], f32, kind="ExternalOutput").ap()

    with tile.TileContext(nc) as tc:
        with (
            tc.tile_pool(name="const", bufs=1) as cpool,
            tc.tile_pool(name="resid", bufs=1) as rpool,
            tc.tile_pool(name="xcol", bufs=4) as xpool,
            tc.tile_pool(name="ptile", bufs=4) as ppool,
            tc.tile_pool(name="ptsb", bufs=10) as ptpool,
            tc.tile_pool(name="tmp", bufs=10) as tpool,
            tc.tile_pool(name="stat", bufs=40) as spool,
            tc.tile_pool(name="obuf", bufs=3) as opool,
            tc.tile_pool(name="ps4", bufs=1, space="PSUM") as ps4,
            tc.tile_pool(name="ps1", bufs=5, space="PSUM") as ps1,
        ):
            # ---- constants / weights resident in SBUF ----
            wq_sb = cpool.tile([P, DT, NH * DH], md, tag="wq")
            nc.sync.dma_start(wq_sb[:], wqT_d.rearrange("(dt p) e -> p dt e", p=P))
            wkv_sb = cpool.tile([P, DT, 2 * DH], md, tag="wkv")
            nc.sync.dma_start(wkv_sb[:], wkvT_d.rearrange("(dt p) e -> p dt e", p=P))
            wo_sb = cpool.tile([P, NH * DH // P, D], md, tag="wo")
            nc.sync.dma_start(wo_sb[:], woT2_d.rearrange("(et p) d1 -> p et d1", p=P))
            cos_sb = cpool.tile([P, TT, DH], md, tag="cos")
            nc.sync.dma_start(cos_sb[:], cos_d.rearrange("(tt p) d1 -> p tt d1", p=P))
            sin_sb = cpool.tile([P, TT, DH], md, tag="sin")
            nc.sync.dma_start(sin_sb[:], sin_d.rearrange("(tt p) d1 -> p tt d1", p=P))
            qw_sb = cpool.tile([P, DH], f32, tag="qw")
            nc.sync.dma_start(qw_sb[:], qw_d)
            kw_sb = cpool.tile([P, DH], f32, tag="kw")
            nc.sync.dma_start(kw_sb[:], kw_d)
            tri_sb = cpool.tile([P, P], f32, tag="tri")
            nc.sync.dma_start(tri_sb[:], trimask_d)
            ident = cpool.tile([P, P], md, tag="ident")
            nc.sync.dma_start(ident[:], ident_d)
            eps_sb = cpool.tile([P, 1], f32, tag="eps")
            nc.gpsimd.memset(eps_sb[:], EPS)

            # ---- persistent activations ----
            qT_sb = rpool.tile([P, NH * 2, S], md, tag="qT")   # [dh-part, h*2+dh, t]
            kT_sb = rpool.tile([P, 2, S], md, tag="kT")
            v_sb = rpool.tile([P, TT, DH], md, tag="v")        # token-major
            aT_sb = rpool.tile([P, NH * 2, S], md, tag="aT")   # attnT

            xT_r = xT_d.rearrange("(dt p) t -> p dt t", p=P)

            env = dict(
                f32=f32, md=md, Alu=Alu, Act=Act, X=X, XY=XY,
                wq_sb=wq_sb, wkv_sb=wkv_sb, wo_sb=wo_sb, cos_sb=cos_sb,
                sin_sb=sin_sb, qw_sb=qw_sb, kw_sb=kw_sb, tri_sb=tri_sb,
                ident=ident, eps_sb=eps_sb, qT_sb=qT_sb, kT_sb=kT_sb,
                v_sb=v_sb, aT_sb=aT_sb, xT_r=xT_r, out_d=out_d,
                xpool=xpool, ppool=ppool, ptpool=ptpool, tpool=tpool,
                spool=spool, opool=opool, ps4=ps4, ps1=ps1,
            )
            import contextlib
            loop_ctx = (tc.For_i(0, ITERS, 1) if ITERS > 1
                        else contextlib.nullcontext())
            with loop_ctx:
                _emit_body(nc, tc, env)

    nc.compile()
    return nc


def _emit_body(nc, tc, env):
    f32, md = env["f32"], env["md"]
    Alu, Act, X, XY = env["Alu"], env["Act"], env["X"], env["XY"]
    wq_sb, wkv_sb, wo_sb = env["wq_sb"], env["wkv_sb"], env["wo_sb"]
    cos_sb, sin_sb = env["cos_sb"], env["sin_sb"]
    qw_sb, kw_sb, tri_sb = env["qw_sb"], env["kw_sb"], env["tri_sb"]
    ident, eps_sb = env["ident"], env["eps_sb"]
    qT_sb, kT_sb, v_sb, aT_sb = env["qT_sb"], env["kT_sb"], env["v_sb"], env["aT_sb"]
    xT_r, out_d = env["xT_r"], env["out_d"]
    xpool, ppool, ptpool = env["xpool"], env["ppool"], env["ptpool"]
    tpool, spool, opool = env["tpool"], env["spool"], env["opool"]
    ps4, ps1 = env["ps4"], env["ps1"]

    # ==== proj phase, then attention with big/small tiles interleaved ====
    for tt in range(TT):
        _emit_proj_tile(nc, tc, env, tt)
    order = []
    lo, hi = 0, TT - 1
    while lo <= hi:
        order.append(hi)
        if lo < hi:
            order.append(lo)
        hi -= 1
        lo += 1
    for i in order:
        _emit_attn_tile(nc, tc, env, i)


def _emit_proj_tile(nc, tc, env, tt):
    f32, md = env["f32"], env["md"]
    Alu, Act, X, XY = env["Alu"], env["Act"], env["X"], env["XY"]
    wq_sb, wkv_sb = env["wq_sb"], env["wkv_sb"]
    cos_sb, sin_sb = env["cos_sb"], env["sin_sb"]
    qw_sb, kw_sb = env["qw_sb"], env["kw_sb"]
    ident, eps_sb = env["ident"], env["eps_sb"]
    qT_sb, kT_sb, v_sb = env["qT_sb"], env["kT_sb"], env["v_sb"]
    xT_r = env["xT_r"]
    xpool, tpool, spool = env["xpool"], env["tpool"], env["spool"]
    ps1 = env["ps1"]

    if True:
        xcol = xpool.tile([P, DT, P], md, tag="xcol")
        nc.sync.dma_start(xcol[:], xT_r[:, :, tt * P:(tt + 1) * P])
        q_ps = ps1.tile([P, NH * DH], f32, tag="work")
        kv_ps = ps1.tile([P, 2 * DH], f32, tag="work")
        for d in range(DT):
            nc.tensor.matmul(q_ps[:], xcol[:, d, :], wq_sb[:, d, :],
                             start=(d == 0), stop=(d == DT - 1))
        for d in range(DT):
            nc.tensor.matmul(kv_ps[:], xcol[:, d, :], wkv_sb[:, d, :],
                             start=(d == 0), stop=(d == DT - 1))

        # ---- q/k: rms-norm + weight + rope (token-major), then transpose
        tp_ps = ps1.tile([P, 512], md, tag="work")  # 4 transpose blocks
        hd = DH // 2
        ct = cos_sb[:, tt, :]
        st = sin_sb[:, tt, :]
        # norm statistics in two independent pairs: (q0,q1) and (k,v)
        srcs = [q_ps[:, 0:DH], q_ps[:, DH:2 * DH], kv_ps[:, 0:DH],
                kv_ps[:, DH:2 * DH]]
        rrs = []
        for pair in (0, 1):
            ss2 = spool.tile([P, 2], f32, tag=f"ss{pair}", name="ss2")
            for j in (0, 1):
                sq = tpool.tile([P, DH], f32, tag="sq")
                nc.scalar.activation(sq[:], srcs[2 * pair + j], Act.Square,
                                     accum_out=ss2[:, j:j + 1])
            rt2 = spool.tile([P, 2], f32, tag=f"rt{pair}", name="rt2")
            nc.scalar.activation(rt2[:], ss2[:], Act.Sqrt,
                                 bias=eps_sb[:], scale=1.0 / DH)
            rr2 = spool.tile([P, 2], f32, tag=f"rr{pair}", name="rr2")
            nc.vector.reciprocal(rr2[:], rt2[:])
            rrs.append(rr2)
        rr_of = [rrs[0][:, 0:1], rrs[0][:, 1:2], rrs[1][:, 0:1], rrs[1][:, 1:2]]
        for which in range(NH + 1):  # 0,1 = q heads; 2 = k
            if which < NH:
                src = q_ps[:, which * DH:(which + 1) * DH]
                wvec = qw_sb
            else:
                src = kv_ps[:, 0:DH]
                wvec = kw_sb
            # qa = (src * rr) * w
            qa = tpool.tile([P, DH], md, tag="qa")
            nc.vector.scalar_tensor_tensor(
                qa[:], src, rr_of[which], wvec[:],
                op0=Alu.mult, op1=Alu.mult)
            # rope (all fp16, 2x DVE mode)
            qr = tpool.tile([P, DH], md, tag="qr")
            t1 = tpool.tile([P, hd], md, tag="t1")
            t2 = tpool.tile([P, hd], md, tag="t2")
            nc.vector.tensor_mul(t1[:], qa[:, 0:hd], ct[:, 0:hd])
            nc.vector.tensor_mul(t2[:], qa[:, hd:DH], st[:, 0:hd])
            nc.vector.tensor_sub(qr[:, 0:hd], t1[:], t2[:])
            t3 = tpool.tile([P, hd], md, tag="t1")
            t4 = tpool.tile([P, hd], md, tag="t2")
            nc.vector.tensor_mul(t3[:], qa[:, hd:DH], ct[:, hd:DH])
            nc.vector.tensor_mul(t4[:], qa[:, 0:hd], st[:, hd:DH])
            nc.vector.tensor_add(qr[:, hd:DH], t3[:], t4[:])
            # transpose both dh halves into head-major layout
            for dh in range(2):
                nc.tensor.transpose(
                    tp_ps[:, ((2 * which + dh) % 4) * P:((2 * which + dh) % 4 + 1) * P],
                    qr[:, dh * P:(dh + 1) * P], ident[:])
            if which == 1:
                # q heads 0,1 -> 4 transposed blocks, one batched copy
                nc.vector.tensor_copy(
                    qT_sb[:, :, tt * P:(tt + 1) * P],
                    tp_ps[:].rearrange("p (b q1) -> p b q1", b=4))
                tp_ps = ps1.tile([P, 512], md, tag="work")
            elif which == 2:
                nc.vector.tensor_copy(
                    kT_sb[:, :, tt * P:(tt + 1) * P],
                    tp_ps[:, 0:2 * P].rearrange("p (b q1) -> p b q1", b=2))

        # ---- v: rms-norm only, stays token-major
        vsrc = kv_ps[:, DH:2 * DH]
        nc.vector.tensor_scalar_mul(v_sb[:, tt, :], vsrc, rr_of[3])


def _emit_attn_tile(nc, tc, env, i):
    f32, md = env["f32"], env["md"]
    Alu, Act, X, XY = env["Alu"], env["Act"], env["X"], env["XY"]
    wo_sb, tri_sb, ident = env["wo_sb"], env["tri_sb"], env["ident"]
    qT_sb, kT_sb, v_sb, aT_sb = env["qT_sb"], env["kT_sb"], env["v_sb"], env["aT_sb"]
    out_d = env["out_d"]
    ppool, ptpool = env["ppool"], env["ptpool"]
    tpool, spool, opool = env["tpool"], env["spool"], env["opool"]
    ps4, ps1 = env["ps4"], env["ps1"]

    ET = NH * DH // P  # 4
    if True:
        W = i // 4 + 1        # active key chunks of 512
        m = i % 4             # partial block index in the diagonal chunk
        wd = (m + 1) * P      # live width of the diagonal chunk
        nlive = i + 1         # live 128-key blocks
        for h in range(NH):
            # full chunks go to the 3-bank score tile; diagonal chunk to its
            # own 1-bank tile so the next tile's scores can start earlier
            s_ps = (ps4.tile([P, 3, KC], f32, tag="score", name="s_ps")
                    if W > 1 else None)
            d_ps = ps1.tile([P, KC], f32, tag="work")
            for dh in range(2):
                lhsT = qT_sb[:, h * 2 + dh, i * P:(i + 1) * P]
                for kc in range(W - 1):
                    nc.tensor.matmul(
                        s_ps[:, kc, :], lhsT,
                        kT_sb[:, dh, kc * KC:(kc + 1) * KC],
                        start=(dh == 0), stop=(dh == 1))
                nc.tensor.matmul(
                    d_ps[:, 0:wd], lhsT,
                    kT_sb[:, dh, (W - 1) * KC:(W - 1) * KC + wd],
                    start=(dh == 0), stop=(dh == 1))
            # causal mask on the triangular block only
            nc.vector.tensor_add(d_ps[:, m * P:wd], d_ps[:, m * P:wd],
                                 tri_sb[:])
            # row max over live region (negated for the exp bias)
            negm = spool.tile([P, 1], f32, tag="negm")
            if W > 1:
                nm1 = spool.tile([P, 1], f32, tag="nm1")
                nc.vector.tensor_reduce(nm1[:], s_ps[:, 0:W - 1, :], axis=XY,
                                        op=Alu.max, negate=True)
                nm2 = spool.tile([P, 1], f32, tag="nm2")
                nc.vector.tensor_reduce(nm2[:], d_ps[:, 0:wd], axis=X,
                                        op=Alu.max, negate=True)
                nc.vector.tensor_tensor(negm[:], nm1[:], nm2[:], op=Alu.min)
            else:
                nc.vector.tensor_reduce(negm[:], d_ps[:, 0:wd], axis=X,
                                        op=Alu.max, negate=True)
            # exp + sums (full chunks in one op, partial chunk in another)
            p_sb = ppool.tile([P, 4, KC], md, tag="p")
            zs = spool.tile([P, 2], f32, tag="zs")
            if W > 1:
                nc.scalar.activation(p_sb[:, 0:W - 1, :], s_ps[:, 0:W - 1, :],
                                     Act.Exp, bias=negm[:],
                                     accum_out=zs[:, 0:1])
            else:
                nc.gpsimd.memset(zs[:, 0:1], 0.0)
            nc.scalar.activation(p_sb[:, W - 1, 0:wd], d_ps[:, 0:wd],
                                 Act.Exp, bias=negm[:], accum_out=zs[:, 1:2])
            z = spool.tile([P, 1], f32, tag="z")
            nc.vector.reduce_sum(z[:], zs[:], axis=X)

            # transpose live prob blocks (packed 4 per psum bank)
            ngroups = (nlive + 3) // 4
            pt_sbs = []
            for g in range(ngroups):
                blocks = range(g * 4, min(g * 4 + 4, nlive))
                ptp = ps1.tile([P, 512], md, tag="work")
                for lb in blocks:
                    j = lb - g * 4
                    nc.tensor.transpose(
                        ptp[:, j * P:(j + 1) * P],
                        p_sb[:, lb // 4, (lb % 4) * P:(lb % 4 + 1) * P],
                        ident[:])
                nbl = len(blocks)
                pt_sb = ptpool.tile([P, 512], md, tag="pt")
                nc.vector.tensor_copy(pt_sb[:, 0:nbl * P], ptp[:, 0:nbl * P])
                pt_sbs.append(pt_sb)
            # PV over live blocks
            a_ps = ps1.tile([P, DH], f32, tag="work")
            for lb in range(nlive):
                nc.tensor.matmul(
                    a_ps[:], pt_sbs[lb // 4][:, (lb % 4) * P:(lb % 4 + 1) * P],
                    v_sb[:, lb, :],
                    start=(lb == 0), stop=(lb == nlive - 1))
            # normalize + store attnT
            rz = spool.tile([P, 1], f32, tag="rz")
            nc.vector.reciprocal(rz[:], z[:])
            at = tpool.tile([P, DH], md, tag="at")
            nc.vector.tensor_scalar_mul(at[:], a_ps[:], rz[:])
            atp = ps1.tile([P, 512], md, tag="work")
            for e in range(2):
                nc.tensor.transpose(atp[:, e * P:(e + 1) * P],
                                    at[:, e * P:(e + 1) * P], ident[:])
            nc.vector.tensor_copy(
                aT_sb[:, h * 2:h * 2 + 2, i * P:(i + 1) * P],
                atp[:, 0:2 * P].rearrange("p (b q1) -> p b q1", b=2))

        # ---- output projection for this q-tile (fills PE gaps) ----
        for dc in range(D // KC):  # 4 chunks of 512
            o_ps = ps1.tile([P, KC], f32, tag="work")
            for e in range(ET):
                nc.tensor.matmul(
                    o_ps[:], aT_sb[:, e, i * P:(i + 1) * P],
                    wo_sb[:, e, dc * KC:(dc + 1) * KC],
                    start=(e == 0), stop=(e == ET - 1))
            o_sb = opool.tile([P, KC], f32, tag="o")
            nc.scalar.copy(o_sb[:], o_ps[:])
            nc.sync.dma_start(
                out_d[i * P:(i + 1) * P, dc * KC:(dc + 1) * KC], o_sb[:])


def _host_prep(inputs):
    """Build the 8 per-core input maps from full inputs."""
    x = np.asarray(inputs["hidden_states"], np.float32)
    cos = np.asarray(inputs["cos"], np.float32)
    sin = np.asarray(inputs["sin"], np.float32)
    wq = np.asarray(inputs["wq"], np.float32)
    wk = np.asarray(inputs["wk"], np.float32)
    wv = np.asarray(inputs["wv"], np.float32)
    wo = np.asarray(inputs["wo"], np.float32)
    qnw = np.asarray(inputs["q_norm_w"], np.float32)
    knw = np.asarray(inputs["k_norm_w"], np.float32)

    md = _np_md()
    qw_b = np.ascontiguousarray(np.broadcast_to(qnw, (P, DH))).astype(np.float32)
    kw_b = np.ascontiguousarray(np.broadcast_to(knw, (P, DH))).astype(np.float32)

    # additive lower-triangular mask for the diagonal 128x128 block
    r = np.arange(P)[:, None]
    c = np.arange(P)[None, :]
    trimask = np.where(c <= r, 0.0, NEG).astype(np.float32)

    xT = [np.ascontiguousarray(x[b].T).astype(md) for b in range(B)]

    in_maps = []
    for cid in range(8):
        b = cid // 4
        j = cid % 4
        h0 = 2 * j
        g = j // 2
        wqT = np.ascontiguousarray(wq[h0 * DH:(h0 + 2) * DH, :].T).astype(md)
        wkvT = np.ascontiguousarray(
            np.concatenate([wk[g * DH:(g + 1) * DH, :],
                            wv[g * DH:(g + 1) * DH, :]], axis=0).T).astype(md)
        woT2 = np.ascontiguousarray(wo[:, h0 * DH:(h0 + 2) * DH].T).astype(md)
        def v2(a):
            return a.view(np.uint16) if a.dtype.itemsize == 2 else a
        in_maps.append({
            "xT": v2(xT[b]),
            "wqT": v2(wqT),
            "wkvT": v2(wkvT),
            "woT2": v2(woT2),
            "cosb": v2(np.ascontiguousarray(cos[b]).astype(md)),
            "sinb": v2(np.ascontiguousarray(sin[b]).astype(md)),
            "qw": qw_b,
            "kw": kw_b,
            "trimask": trimask,
            "ident": v2(np.eye(P, dtype=md)),
        })
    return in_maps


def kernel(**inputs) -> np.ndarray:
    if "nc" not in _cache:
        _cache["nc"] = _build_program()
    nc = _cache["nc"]
    in_maps = _host_prep(inputs)
    res = bass_utils.run_bass_kernel_spmd(
        nc, in_maps, core_ids=list(range(8)))
    _cache["last_result"] = res
    out = np.zeros((B, S, D), np.float32)
    for cid in range(8):
        out[cid // 4] += res.results[cid]["out"]
    return out

